# revision 44
# baseline (speedup 1.0000x reference)
"""Multi-head self-attention (B=4, L=2048, D=512, H=4, Hd=128) on 8 TRN2 cores.

Sharding: core c handles batch b = c//2 and head-pair p = c%2 (heads 2p, 2p+1).
Each core computes a partial output y_part[b] = sum_{h in pair} ctx_h @ Wo_h.T;
host gathers: y[b] = y_part[core 2b] + y_part[core 2b+1] + bo.

Dataflow per core (matmuls bf16 inputs, fp32 PSUM accumulation):
  xT [512,2048] (host-pretransposed)  ->  QT,KT [hd,L] and V [L,hd] via PE;
  the projection windows chase the per-window x DMA stream from inside the
  first two attention units' slots.
  scoresT [k,L_q] = KT_blk.T @ QT     (k-major: softmax along free dim never
  attnT = exp(scoresT/sqrt(hd))        needs a transpose anywhere)
  ctxT [hd,L_q] += V_blk.T @ attnT    (accumulate over k blocks, UNnormalized)
  rowsum: 4-level bf16 fold tree (DVE + one self-contained gpsimd subtree
  per unit) -> ONE rank-1 matmul/unit -> row [1,512] -> DVE copy -> DRAM
  bounce -> transposed read [128,4] -> DVE reciprocal (emitted two slots
  after the bounce so the strict DVE FIFO never parks on its latency).
  For the LAST TWO units the rowsum goes straight to column layout via
  tiny stationary-sm4 matmuls (no DRAM bounce on the critical tail).
  Warmup: 8 N=512 matmuls in ONE PSUM accumulation group (no sem chain)
  ramp the PE clock while the input DMA streams.
  outproj per 128-row window: two separate head matmuls (unnormalized ct),
  then the softmax normalization is applied as per-partition scalars during
  the PSUM drain:  ysb = (ps_h0 * rcp0) + (ps_h1 * rcp1)  via DVE TS + STT.
  HW rule found the hard way: only ONE open (start-without-stop) PSUM
  accumulation group per bank at a time.
"""
import numpy as np
import ml_dtypes

B, L, D = 4, 2048, 512
H, HD = 4, 128
NCORES = 8
QW = 512          # query window (matmul N / PSUM bank pair)
NQC = L // QW     # 4 query windows
NKB = L // 128    # 16 key blocks
NDC = D // 128    # 4 contraction chunks for projections
SCALE = 1.0 / np.sqrt(HD)

_COMPILED = None


def _build():
    import concourse.bass as bass
    import concourse.mybir as mybir
    import concourse.tile as tile
    from concourse import bacc

    F32 = mybir.dt.float32
    F32R = mybir.dt.float32r
    BF16 = mybir.dt.bfloat16
    F8 = mybir.dt.float8e4
    PM = mybir.MatmulPerfMode.DoubleRow
    AF = mybir.ActivationFunctionType
    ALU = mybir.AluOpType

    nc = bacc.Bacc("TRN2", target_bir_lowering=False, debug=False,
                   num_devices=NCORES)
    xT_d = nc.dram_tensor("xT", [D, L], BF16, kind="ExternalInput")
    wqT_d = nc.dram_tensor("wqT", [D, 256], BF16, kind="ExternalInput")
    wkT_d = nc.dram_tensor("wkT", [D, 256], BF16, kind="ExternalInput")
    wvT_d = nc.dram_tensor("wvT", [D, 256], BF16, kind="ExternalInput")
    woT_d = nc.dram_tensor("woT", [256, D], BF16, kind="ExternalInput")
    bqk_d = nc.dram_tensor("bqk", [128, 4], F32, kind="ExternalInput")
    bv_d = nc.dram_tensor("bv", [1, 256], F32, kind="ExternalInput")
    y_d = nc.dram_tensor("y", [L, D], F32, kind="ExternalOutput")

    with tile.TileContext(nc) as tc:
        with (
            tc.tile_pool(name="singles", bufs=1) as singles,
            tc.tile_pool(name="pss", bufs=2, space="PSUM") as pss_pool,
            tc.tile_pool(name="psc", bufs=1, space="PSUM") as psc_pool,
            tc.tile_pool(name="psr", bufs=1, space="PSUM") as psr_pool,
            tc.tile_pool(name="psy", bufs=1, space="PSUM") as psy_pool,
            tc.tile_pool(name="attnp", bufs=16) as attnp,
            tc.tile_pool(name="smf", bufs=6) as smfp,
            tc.tile_pool(name="sm2", bufs=8) as sm2p,
            tc.tile_pool(name="sm3", bufs=6) as sm3p,
            tc.tile_pool(name="rrp", bufs=2) as rrp,
            tc.tile_pool(name="rcl", bufs=2) as rclp,
            tc.tile_pool(name="tmpp", bufs=3) as tmpp,
            tc.tile_pool(name="yp", bufs=3) as yp,
            tc.tile_pool(name="drp", bufs=2, space="DRAM") as drp,
        ):
            gp, sc, sy = nc.gpsimd, nc.scalar, nc.sync

            # memsets on gpsimd: it runs earliest after the boot barrier, so
            # the warmup matmuls (which depend on these) start ~1us sooner
            ones_sb = singles.tile([128, 1], BF16)
            nc.gpsimd.memset(ones_sb[:], 1.0)
            warm_sb = singles.tile([128, 512], BF16)
            warmw_sb = singles.tile([128, 128], BF16)
            nc.gpsimd.memset(warm_sb[:], 0.0)
            nc.gpsimd.memset(warmw_sb[:], 0.0)

            # ---- input loads spread over FOUR trigger queues (gp/sc/sy/vec):
            # per-queue DMA streams top out ~70-136 GB/s, so more queues =
            # earlier arrival.  Per-queue order puts the x window chunks
            # FIRST (the first projection is gated on x w0 + wk h0), weights
            # interleaved behind them.  Weights split in dc-halves so each
            # projection chunk-matmul is gated only by its own half.
            def w_half(d, half):
                a = d.ap()
                return bass.AP(tensor=a.tensor, offset=half * 2 * 128 * 256,
                               ap=[[256, 128], [128 * 256, 2], [1, 256]])

            wq_sb = singles.tile([128, NDC, 256], BF16)
            wk_sb = singles.tile([128, NDC, 256], BF16)
            wv_sb = singles.tile([128, NDC, 256], BF16)
            xt_sb = singles.tile([128, NDC, NQC, QW], BF16)
            bqk_sb = singles.tile([128, 4], F32)
            bv_sb = singles.tile([128, 256], F32)
            wo_sb = singles.tile([128, 2, D], BF16)
            def x_load(q, w, dc):
                q.dma_start(xt_sb[:, dc, w, :],
                            xT_d[128 * dc:128 * dc + 128,
                                 QW * w:QW * w + QW])

            # per-queue programs, ordered by first-use deadline; each queue
            # carries ~8x 128KB so transfer streams stay balanced
            x_load(gp, 0, 0)
            x_load(sc, 0, 1)
            x_load(sy, 0, 2)
            gp.dma_start(wq_sb[:, 0:2, :], w_half(wqT_d, 0))
            sc.dma_start(wk_sb[:, 0:2, :], w_half(wkT_d, 0))
            x_load(sy, 0, 3)
            sc.dma_start(bqk_sb[:], bqk_d[:])
            sy.dma_start(wk_sb[:, 2:4, :], w_half(wkT_d, 1))
            x_load(gp, 1, 0)
            # bv on sc (HWDGE): its 1KB->128KB replicated write previously
            # sat mid-queue on sy and pushed sy's later x chunks past 19us
            sc.dma_start(
                bv_sb[:],
                bass.AP(tensor=bv_d.ap().tensor, offset=0,
                        ap=[[0, 128], [1, 256]]))
            x_load(sy, 1, 2)
            gp.dma_start(wv_sb[:, 0:2, :], w_half(wvT_d, 0))
            x_load(sc, 1, 1)
            sc.dma_start(wq_sb[:, 2:4, :], w_half(wqT_d, 1))
            x_load(sc, 1, 3)
            sy.dma_start(wv_sb[:, 2:4, :], w_half(wvT_d, 1))

            def x_load2(q, dc):
                # windows 2+3 merged: 2KB contiguous source rows halve the
                # descriptor count -> better per-queue DMA rate on the tail
                q.dma_start(xt_sb[:, dc, 2:4, :],
                            xT_d[128 * dc:128 * dc + 128, 2 * QW:4 * QW])

            x_load2(gp, 0)
            x_load2(sc, 1)
            x_load2(sy, 2)
            x_load(gp, 2, 3)
            x_load(sy, 3, 3)
            gp.dma_start(wo_sb[:, 0, :], woT_d[0:128, :])
            sc.dma_start(wo_sb[:, 1, :], woT_d[128:256, :])

            # PE warmup while input DMA streams: ramps the PE p-state so the
            # first real matmuls run at full clock.  One accumulation group
            # -> the warmup matmuls issue back-to-back with no sem chain.
            ps_w = psy_pool.tile([128, 2, 512], F32, name="ps_w", tag="psy")
            NWARM = 8
            for wi in range(NWARM):
                nc.tensor.matmul(ps_w[:, 0, :], warmw_sb[:], warm_sb[:],
                                 start=(wi == 0), stop=(wi == NWARM - 1))

            qt_sb = singles.tile([128, 2, L], BF16)   # QT per head [hd, L]
            kt_sb = singles.tile([128, 2, L], BF16)
            v_sb = singles.tile([128, NKB, 256], BF16)
            ct = [singles.tile([128, QW], BF16, name=f"ct{t}")
                  for t in range(2 * NQC)]
            rcp = [singles.tile([128, NQC], F32, name=f"rcp{t}")
                   for t in range(2 * NQC)]

            def emit_proj(o_sb, w_sb, h, qc, bcol, pool, tag):
                win = slice(QW * qc, QW * qc + QW)
                ps = pool.tile([128, QW], F32, name=f"pp{tag}{h}{qc}", tag=tag)
                for dc in range(NDC):
                    nc.tensor.matmul(ps[:], w_sb[:, dc, 128 * h:128 * h + 128],
                                     xt_sb[:, dc, qc, :],
                                     start=(dc == 0), stop=(dc == NDC - 1))
                nc.vector.tensor_scalar_add(
                    o_sb[:, h, win], ps[:], bqk_sb[:, bcol:bcol + 1])

            def emit_vproj(lb, pool, tag):
                ps = pool.tile([128, QW], F32, name=f"pv{lb}", tag=tag)
                for dc in range(NDC):
                    nc.tensor.matmul(
                        ps[:, 0:256],
                        xt_sb[:, dc, lb // 4, 128 * (lb % 4):128 * (lb % 4) + 128],
                        wv_sb[:, dc, :],
                        start=(dc == 0), stop=(dc == NDC - 1))
                nc.vector.tensor_add(v_sb[:, lb, :], ps[:, 0:256], bv_sb[:])

            # ---- per-unit state ----
            # unit t = (qc, h) with t = 2*qc + h; scores/exp of unit t overlap
            # ctx of unit t-1 (cross-iteration software pipeline).
            smf = {}   # (t, kk) -> [128,512] pair-fold
            sm2 = {}
            sm3 = {}
            sm4 = {}

            def emit_scores_pair(t, qc, h, kk):
                win = slice(QW * qc, QW * qc + QW)
                ps_s = pss_pool.tile([128, 1024], F32,
                                     name=f"ps_s{t}_{kk}", tag="pss")
                k0 = 256 * kk
                nc.tensor.matmul(ps_s[:, 0:512], kt_sb[:, h, k0:k0 + 128],
                                 qt_sb[:, h, win], start=True, stop=True)
                nc.tensor.matmul(ps_s[:, 512:1024],
                                 kt_sb[:, h, k0 + 128:k0 + 256],
                                 qt_sb[:, h, win], start=True, stop=True)
                at = attnp.tile([128, 1024], BF16, name=f"at{t}_{kk}",
                                tag="attn")
                nc.scalar.activation(at[:], ps_s[:], AF.Exp, scale=SCALE)
                sm = smfp.tile([128, 512], BF16, name=f"sm{t}_{kk}", tag="sm")
                # gpsimd takes ONE self-contained subtree per unit (kk 0-1
                # lvl1 + their lvl2).  Larger gpsimd shares regressed hard:
                # its TT is ~2x slower, and the rowsum chain ends in a PE
                # rank-1 matmul sitting in the strict PE FIFO — any gpsimd
                # lag there stalls the ENTIRE matmul stream.
                fe = nc.gpsimd if kk in (0, 1) else nc.vector
                fe.tensor_add(sm[:], at[:, 0:512], at[:, 512:1024])
                smf[(t, kk)] = sm
                # second fold level: 4 rank-1 inputs per unit
                if kk % 2 == 1:
                    s2 = sm2p.tile([128, 512], BF16, name=f"s2{t}_{kk}",
                                   tag="sm2")
                    fe.tensor_add(s2[:], smf[(t, kk - 1)][:], sm[:])
                    sm2[(t, kk // 2)] = s2
                return at

            def emit_fold3(t, jj):
                # third fold level (DVE): sm3 = sm2[2jj] + sm2[2jj+1]
                s3 = sm3p.tile([128, 512], BF16, name=f"s3{t}_{jj}",
                               tag="sm3")
                nc.vector.tensor_add(s3[:], sm2[(t, 2 * jj)][:],
                                     sm2[(t, 2 * jj + 1)][:])
                sm3[(t, jj)] = s3

            def emit_fold4(t):
                # fourth fold level: one [128,512] tile holds the full
                # 16-block key fold -> a SINGLE rank-1 matmul per unit.
                s4 = sm3p.tile([128, 512], BF16, name=f"s4{t}", tag="sm3")
                nc.vector.tensor_add(s4[:], sm3[(t, 0)][:], sm3[(t, 1)][:])
                sm4[t] = s4

            def emit_rank1(t, ps_r):
                nc.tensor.matmul(ps_r[:], ones_sb[:], sm4[t][:],
                                 start=True, stop=True)

            def emit_rank1T(t, qq, ps_rc):
                # rowsum directly in column layout [128,4]: tiny
                # stationary-sm4 matmul per query window. Used for the last
                # two units only, where the DRAM-bounce transpose latency
                # would sit on the critical tail.
                nc.tensor.matmul(ps_rc[:, qq:qq + 1],
                                 sm4[t][:, 128 * qq:128 * qq + 128],
                                 ones_sb[:], start=True, stop=True)

            def emit_ctx_pair(st, kk):
                t, h, at_tiles, ps_c, _ = st
                hs = slice(128 * h, 128 * h + 128)
                at = at_tiles[kk]
                nc.tensor.matmul(ps_c[:], v_sb[:, 2 * kk, hs], at[:, 0:512],
                                 start=(kk == 0), stop=False)
                nc.tensor.matmul(ps_c[:], v_sb[:, 2 * kk + 1, hs],
                                 at[:, 512:1024], start=False,
                                 stop=(kk == NKB // 2 - 1))

            def emit_recip(t, ps_rc):
                nc.vector.reciprocal(rcp[t][:], ps_rc[:])

            rcol_t = {}

            def emit_rowsum_bounce(t, ps_r):
                # ps_r [1,512] row -> DRAM -> transposed read [128,4]; the
                # reciprocal is emitted separately TWO slots later so the
                # DVE FIFO never parks on the bounce's ~2-3us latency.
                # row copy on DVE (short PE dep; keeps the saturated ACT
                # queue out of the rowsum chain); bounce DMAs on gp, which
                # carries no bulk transfers mid-stream
                rrow = rrp.tile([1, QW], F32, name=f"rr{t}", tag="rr")
                nc.vector.tensor_copy(rrow[:], ps_r[:])
                rd = drp.tile([1, QW], F32, name=f"rd{t}", tag="rd")
                gp.dma_start(rd[:], rrow[:])
                rda = rd[:]
                rcol = rclp.tile([128, NQC], F32, name=f"rc{t}", tag="rc")
                gp.dma_start(rcol[:],
                             bass.AP(tensor=rda.tensor, offset=rda.offset,
                                     ap=[[1, 128], [128, NQC]]))
                rcol_t[t] = rcol

            def emit_rowsum_recip(t):
                nc.vector.reciprocal(rcp[t][:], rcol_t[t][:])

            def emit_ct_copy(t, ps_c, on_act=False):
                if on_act:
                    nc.scalar.activation(ct[t][:], ps_c[:], AF.Copy)
                else:
                    nc.vector.tensor_copy(ct[t][:], ps_c[:])

            def emit_outproj_window(qc, qq, pool, tag, tmp_act=False):
                qsl = slice(128 * qq, 128 * qq + 128)
                ps_y = pool.tile([128, 2, 512], F32, name=f"py{qc}_{qq}",
                                 tag=tag)
                nc.tensor.matmul(ps_y[:, 0, :], ct[2 * qc][:, qsl],
                                 wo_sb[:, 0, :], start=True, stop=True)
                nc.tensor.matmul(ps_y[:, 1, :], ct[2 * qc + 1][:, qsl],
                                 wo_sb[:, 1, :], start=True, stop=True)
                tmp = tmpp.tile([128, 512], F32, name=f"tm{qc}_{qq}",
                                tag="tmp")
                if tmp_act:
                    nc.scalar.activation(tmp[:], ps_y[:, 1, :], AF.Copy,
                                         scale=rcp[2 * qc + 1][:, qq:qq + 1])
                else:
                    nc.vector.tensor_scalar_mul(
                        tmp[:], ps_y[:, 1, :], rcp[2 * qc + 1][:, qq:qq + 1])
                ysb = yp.tile([128, 512], F32, name=f"ysb{qc}_{qq}",
                              tag="ysb")
                nc.vector.scalar_tensor_tensor(
                    ysb[:], ps_y[:, 0, :], rcp[2 * qc][:, qq:qq + 1], tmp[:],
                    ALU.mult, ALU.add)
                # y windows go on sync (idle mid-stream; sc would delay the
                # ACT exp queue); the final group splits each window into
                # half-partition transfers on two queues so the tail
                # transfers run in parallel instead of 1.9us serial each
                ywin = y_d[QW * qc + 128 * qq:QW * qc + 128 * qq + 128, :]
                if tmp_act:
                    q0, q1 = (sy, gp) if qq % 2 == 0 else (gp, sy)
                    q0.dma_start(ywin[0:64, :], ysb[0:64, :])
                    q1.dma_start(ywin[64:128, :], ysb[64:128, :])
                else:
                    sy.dma_start(ywin, ysb[:])

            # ---- prologue: just K/Q head 0 window 0; the rest of the
            # projections chase the x DMA stream from inside units 0-1 ----
            emit_proj(kt_sb, wk_sb, 0, 0, 2, pss_pool, "pss")
            emit_proj(qt_sb, wq_sb, 0, 0, 0, pss_pool, "pss")

            # deferred PE work fed into the units' kk slots
            # t=0 fillers use the (still idle) psc bank; t=1 fillers use pss.
            def P(o, w, h, qc, bc, pool, tag):
                return lambda: emit_proj(o, w, h, qc, bc, pool, tag)

            fillers_t0 = [
                P(kt_sb, wk_sb, 0, 1, 2, psc_pool, "psc"),
                P(kt_sb, wk_sb, 0, 2, 2, psc_pool, "psc"),
                P(qt_sb, wq_sb, 0, 1, 0, psc_pool, "psc"),
                P(kt_sb, wk_sb, 0, 3, 2, psc_pool, "psc"),
                P(qt_sb, wq_sb, 0, 2, 0, psc_pool, "psc"),
                P(kt_sb, wk_sb, 1, 0, 3, psc_pool, "psc"),
                P(qt_sb, wq_sb, 1, 0, 1, psc_pool, "psc"),
                P(kt_sb, wk_sb, 1, 1, 3, psc_pool, "psc"),
            ]
            fillers_t1 = [
                P(qt_sb, wq_sb, 0, 3, 0, pss_pool, "pss"),
                P(kt_sb, wk_sb, 1, 2, 3, pss_pool, "pss"),
                P(kt_sb, wk_sb, 1, 3, 3, pss_pool, "pss"),
                P(qt_sb, wq_sb, 1, 1, 1, pss_pool, "pss"),
                P(qt_sb, wq_sb, 1, 2, 1, pss_pool, "pss"),
                P(qt_sb, wq_sb, 1, 3, 1, pss_pool, "pss"),
            ]

            pending_outproj = []   # list of (qc, qq)
            to_pend = []           # windows whose rcp chain is still landing
            prev = None
            T_LAST = 2 * NQC - 1
            for t in range(2 * NQC):
                qc, h = t // 2, t % 2
                ps_c = psc_pool.tile([128, QW], F32, name=f"ps_c{t}",
                                     tag="psc")
                ps_r = (psr_pool.tile([1, QW], F32, name=f"ps_r{t}",
                                      tag="psr") if t < 6 else None)
                at_tiles = []
                sfirst = (t >= 6)   # feed ACT early near the end
                for kk in range(NKB // 2):
                    if sfirst:
                        at_tiles.append(emit_scores_pair(t, qc, h, kk))
                    if prev is not None:
                        emit_ctx_pair(prev, kk)
                        # previous unit's rowsum: rank-1 + drain, placed
                        # late enough that the (DVE) fold chain has finished
                        if prev[0] < 6:
                            if kk == 2:
                                emit_rank1(prev[0], prev[4])
                            elif kk == 3:
                                emit_rowsum_bounce(prev[0], prev[4])
                            elif kk == 5:
                                emit_rowsum_recip(prev[0])
                        else:
                            if kk == 2:
                                prc6 = psr_pool.tile(
                                    [128, NQC], F32, name="prc6", tag="psr")
                                emit_rank1T(6, 0, prc6)
                                emit_rank1T(6, 1, prc6)
                            elif kk == 3:
                                emit_rank1T(6, 2, prc6)
                                emit_rank1T(6, 3, prc6)
                                emit_recip(6, prc6)
                        if kk == 6 and to_pend:
                            pending_outproj.extend(to_pend)
                            to_pend = []
                    if t == 0:
                        emit_vproj(2 * kk, pss_pool, "pss")
                        emit_vproj(2 * kk + 1, pss_pool, "pss")
                        fillers_t0[kk]()
                    elif t == 1 and kk < len(fillers_t1):
                        fillers_t1[kk]()
                    if kk % 2 == 1 and pending_outproj:
                        emit_outproj_window(*pending_outproj.pop(0),
                                            psy_pool, "psy")
                    if not sfirst:
                        at_tiles.append(emit_scores_pair(t, qc, h, kk))
                    if kk == 4:
                        emit_fold3(t, 0)
                emit_fold3(t, 1)
                emit_fold4(t)
                if prev is not None:
                    # finish unit t-1: drain its ctx PSUM to bf16 SBUF
                    emit_ct_copy(prev[0], prev[3])
                    if prev[0] % 2 == 1:
                        # defer the windows to the NEXT unit's kk==6 slot so
                        # their rcp consumers never enter the DVE FIFO
                        # before the rowsum bounce+recip have resolved
                        to_pend = [((prev[0] - 1) // 2, qq)
                                   for qq in range(NQC)]
                prev = (t, h, at_tiles, ps_c, ps_r)

            # ---- drain: ctx of the last unit; its rowsum goes straight to
            # column layout on the PE (no DRAM bounce in the tail) ----
            ps_rc = psr_pool.tile([128, NQC], F32, name="ps_rc", tag="psr")
            for kk in range(NKB // 2):
                emit_ctx_pair(prev, kk)
                # windows deferred past the end of the main loop drain here
                if kk % 2 == 1 and pending_outproj:
                    emit_outproj_window(*pending_outproj.pop(0),
                                        psy_pool, "psy")
                if 2 <= kk <= 5:
                    emit_rank1T(T_LAST, kk - 2, ps_rc)
                    if kk == 5:
                        emit_recip(T_LAST, ps_rc)
            emit_ct_copy(prev[0], prev[3], on_act=True)
            # last window group (qc=3): rotate through pss (now idle) + psy
            for qq in range(NQC):
                # qq3 goes through pss (bufs=2): it then waits on window 0's
                # combine instead of window 1's, starting ~0.7us earlier
                if qq == 1:
                    emit_outproj_window(3, qq, psy_pool, "psy", tmp_act=True)
                else:
                    emit_outproj_window(3, qq, pss_pool, "pss", tmp_act=True)

    nc.compile()
    return nc


def _get_compiled():
    global _COMPILED
    if _COMPILED is None:
        _COMPILED = _build()
    return _COMPILED


def make_in_maps(x, Wq, bq, Wk, bk, Wv, bv, Wo):
    bf16 = ml_dtypes.bfloat16
    xT = {b: np.ascontiguousarray(x[b].T).astype(bf16) for b in range(B)}
    WqT, WkT, WvT, WoT = (np.ascontiguousarray(W.T) for W in (Wq, Wk, Wv, Wo))
    in_maps = []
    for c in range(NCORES):
        b = c // 2
        p = c % 2
        hs = slice(256 * p, 256 * p + 256)
        bqk = np.concatenate(
            [bq[hs].reshape(2, 128).T, bk[hs].reshape(2, 128).T],
            axis=1)
        in_maps.append({
            "xT": xT[b],
            "wqT": WqT[:, hs].astype(bf16),
            "wkT": WkT[:, hs].astype(bf16),
            "wvT": WvT[:, hs].astype(bf16),
            "woT": np.ascontiguousarray(WoT[hs, :]).astype(bf16),
            "bqk": np.ascontiguousarray(bqk, dtype=np.float32),
            "bv": bv[hs].reshape(1, 256).astype(np.float32).copy(),
        })
    return in_maps


def kernel(x, Wq, bq, Wk, bk, Wv, bv, Wo, bo):
    from concourse.bass_utils import run_bass_kernel_spmd

    x = np.asarray(x, np.float32)
    Wq, Wk, Wv, Wo = (np.asarray(w, np.float32) for w in (Wq, Wk, Wv, Wo))
    bq, bk, bv, bo = (np.asarray(b, np.float32) for b in (bq, bk, bv, bo))

    in_maps = make_in_maps(x, Wq, bq, Wk, bk, Wv, bv, Wo)
    nc = _get_compiled()
    try:
        # first execution is a discarded warmup (cold-start timing shakeout);
        # the returned result comes from the second execution
        run_bass_kernel_spmd(nc, in_maps, list(range(NCORES)))
        res = run_bass_kernel_spmd(nc, in_maps, list(range(NCORES)))
    except Exception:
        # one retry: transient device wedges usually clear on re-execution
        res = run_bass_kernel_spmd(nc, in_maps, list(range(NCORES)))
    y = np.empty((B, L, D), np.float32)
    for b in range(B):
        y[b] = res.results[2 * b]["y"] + res.results[2 * b + 1]["y"] + bo
    return y



# revision 45
# speedup vs baseline: 1.0094x; 1.0094x over previous
"""Multi-head self-attention (B=4, L=2048, D=512, H=4, Hd=128) on 8 TRN2 cores.

Sharding: core c handles batch b = c//2 and head-pair p = c%2 (heads 2p, 2p+1).
Each core computes a partial output y_part[b] = sum_{h in pair} ctx_h @ Wo_h.T;
host gathers: y[b] = y_part[core 2b] + y_part[core 2b+1] + bo.

Dataflow per core (matmuls bf16 inputs, fp32 PSUM accumulation):
  xT [512,2048] (host-pretransposed)  ->  QT,KT [hd,L] and V [L,hd] via PE;
  the projection windows chase the per-window x DMA stream from inside the
  first two attention units' slots.
  scoresT [k,L_q] = KT_blk.T @ QT     (k-major: softmax along free dim never
  attnT = exp(scoresT/sqrt(hd))        needs a transpose anywhere)
  ctxT [hd,L_q] += V_blk.T @ attnT    (accumulate over k blocks, UNnormalized)
  rowsum: 4-level bf16 fold tree (DVE + one self-contained gpsimd subtree
  per unit) -> ONE rank-1 matmul/unit -> row [1,512] -> DVE copy -> DRAM
  bounce -> transposed read [128,4] -> DVE reciprocal (emitted two slots
  after the bounce so the strict DVE FIFO never parks on its latency).
  For the LAST TWO units the rowsum goes straight to column layout via
  tiny stationary-sm4 matmuls (no DRAM bounce on the critical tail).
  Warmup: 8 N=512 matmuls in ONE PSUM accumulation group (no sem chain)
  ramp the PE clock while the input DMA streams.
  outproj per 128-row window: two separate head matmuls (unnormalized ct),
  then the softmax normalization is applied as per-partition scalars during
  the PSUM drain:  ysb = (ps_h0 * rcp0) + (ps_h1 * rcp1)  via DVE TS + STT.
  HW rule found the hard way: only ONE open (start-without-stop) PSUM
  accumulation group per bank at a time.
"""
import numpy as np
import ml_dtypes

B, L, D = 4, 2048, 512
H, HD = 4, 128
NCORES = 8
QW = 512          # query window (matmul N / PSUM bank pair)
NQC = L // QW     # 4 query windows
NKB = L // 128    # 16 key blocks
NDC = D // 128    # 4 contraction chunks for projections
SCALE = 1.0 / np.sqrt(HD)

_COMPILED = None


def _build():
    import concourse.bass as bass
    import concourse.mybir as mybir
    import concourse.tile as tile
    from concourse import bacc

    F32 = mybir.dt.float32
    F32R = mybir.dt.float32r
    BF16 = mybir.dt.bfloat16
    F8 = mybir.dt.float8e4
    PM = mybir.MatmulPerfMode.DoubleRow
    AF = mybir.ActivationFunctionType
    ALU = mybir.AluOpType

    nc = bacc.Bacc("TRN2", target_bir_lowering=False, debug=False,
                   num_devices=NCORES)
    xT_d = nc.dram_tensor("xT", [D, L], BF16, kind="ExternalInput")
    wqT_d = nc.dram_tensor("wqT", [D, 256], BF16, kind="ExternalInput")
    wkT_d = nc.dram_tensor("wkT", [D, 256], BF16, kind="ExternalInput")
    wvT_d = nc.dram_tensor("wvT", [D, 256], BF16, kind="ExternalInput")
    woT_d = nc.dram_tensor("woT", [256, D], BF16, kind="ExternalInput")
    bqk_d = nc.dram_tensor("bqk", [128, 4], F32, kind="ExternalInput")
    bv_d = nc.dram_tensor("bv", [1, 256], F32, kind="ExternalInput")
    y_d = nc.dram_tensor("y", [L, D], F32, kind="ExternalOutput")

    with tile.TileContext(nc) as tc:
        with (
            tc.tile_pool(name="singles", bufs=1) as singles,
            tc.tile_pool(name="pss", bufs=2, space="PSUM") as pss_pool,
            tc.tile_pool(name="psc", bufs=1, space="PSUM") as psc_pool,
            tc.tile_pool(name="psr", bufs=1, space="PSUM") as psr_pool,
            tc.tile_pool(name="psy", bufs=1, space="PSUM") as psy_pool,
            tc.tile_pool(name="attnp", bufs=16) as attnp,
            tc.tile_pool(name="smf", bufs=6) as smfp,
            tc.tile_pool(name="sm2", bufs=8) as sm2p,
            tc.tile_pool(name="sm3", bufs=6) as sm3p,
            tc.tile_pool(name="rrp", bufs=2) as rrp,
            tc.tile_pool(name="rcl", bufs=2) as rclp,
            tc.tile_pool(name="tmpp", bufs=4) as tmpp,
            tc.tile_pool(name="yp", bufs=6) as yp,
            tc.tile_pool(name="drp", bufs=2, space="DRAM") as drp,
        ):
            gp, sc, sy = nc.gpsimd, nc.scalar, nc.sync

            # memsets on gpsimd: it runs earliest after the boot barrier, so
            # the warmup matmuls (which depend on these) start ~1us sooner
            ones_sb = singles.tile([128, 1], BF16)
            nc.gpsimd.memset(ones_sb[:], 1.0)
            warm_sb = singles.tile([128, 512], BF16)
            warmw_sb = singles.tile([128, 128], BF16)
            nc.gpsimd.memset(warm_sb[:], 0.0)
            nc.gpsimd.memset(warmw_sb[:], 0.0)

            # ---- input loads spread over FOUR trigger queues (gp/sc/sy/vec):
            # per-queue DMA streams top out ~70-136 GB/s, so more queues =
            # earlier arrival.  Per-queue order puts the x window chunks
            # FIRST (the first projection is gated on x w0 + wk h0), weights
            # interleaved behind them.  Weights split in dc-halves so each
            # projection chunk-matmul is gated only by its own half.
            def w_half(d, half):
                a = d.ap()
                return bass.AP(tensor=a.tensor, offset=half * 2 * 128 * 256,
                               ap=[[256, 128], [128 * 256, 2], [1, 256]])

            wq_sb = singles.tile([128, NDC, 256], BF16)
            wk_sb = singles.tile([128, NDC, 256], BF16)
            wv_sb = singles.tile([128, NDC, 256], BF16)
            xt_sb = singles.tile([128, NDC, NQC, QW], BF16)
            bqk_sb = singles.tile([128, 4], F32)
            bv_sb = singles.tile([128, 256], F32)
            wo_sb = singles.tile([128, 2, D], BF16)
            def x_load(q, w, dc):
                q.dma_start(xt_sb[:, dc, w, :],
                            xT_d[128 * dc:128 * dc + 128,
                                 QW * w:QW * w + QW])

            # per-queue programs, ordered by first-use deadline; each queue
            # carries ~8x 128KB so transfer streams stay balanced
            x_load(gp, 0, 0)
            x_load(sc, 0, 1)
            x_load(sy, 0, 2)
            gp.dma_start(wq_sb[:, 0:2, :], w_half(wqT_d, 0))
            sc.dma_start(wk_sb[:, 0:2, :], w_half(wkT_d, 0))
            x_load(sy, 0, 3)
            sc.dma_start(bqk_sb[:], bqk_d[:])
            sy.dma_start(wk_sb[:, 2:4, :], w_half(wkT_d, 1))
            x_load(gp, 1, 0)
            # bv on sc (HWDGE): its 1KB->128KB replicated write previously
            # sat mid-queue on sy and pushed sy's later x chunks past 19us
            sc.dma_start(
                bv_sb[:],
                bass.AP(tensor=bv_d.ap().tensor, offset=0,
                        ap=[[0, 128], [1, 256]]))
            x_load(sy, 1, 2)
            gp.dma_start(wv_sb[:, 0:2, :], w_half(wvT_d, 0))
            x_load(sc, 1, 1)
            sc.dma_start(wq_sb[:, 2:4, :], w_half(wqT_d, 1))
            x_load(sc, 1, 3)
            sy.dma_start(wv_sb[:, 2:4, :], w_half(wvT_d, 1))

            def x_load2(q, dc):
                # windows 2+3 merged: 2KB contiguous source rows halve the
                # descriptor count -> better per-queue DMA rate on the tail
                q.dma_start(xt_sb[:, dc, 2:4, :],
                            xT_d[128 * dc:128 * dc + 128, 2 * QW:4 * QW])

            x_load2(gp, 0)
            x_load2(sc, 1)
            x_load2(sy, 2)
            x_load(gp, 2, 3)
            x_load(sy, 3, 3)
            gp.dma_start(wo_sb[:, 0, :], woT_d[0:128, :])
            sc.dma_start(wo_sb[:, 1, :], woT_d[128:256, :])

            # PE warmup while input DMA streams: ramps the PE p-state so the
            # first real matmuls run at full clock.  One accumulation group
            # -> the warmup matmuls issue back-to-back with no sem chain.
            ps_w = psy_pool.tile([128, 2, 512], F32, name="ps_w", tag="psy")
            NWARM = 8
            for wi in range(NWARM):
                nc.tensor.matmul(ps_w[:, 0, :], warmw_sb[:], warm_sb[:],
                                 start=(wi == 0), stop=(wi == NWARM - 1))

            qt_sb = singles.tile([128, 2, L], BF16)   # QT per head [hd, L]
            kt_sb = singles.tile([128, 2, L], BF16)
            v_sb = singles.tile([128, NKB, 256], BF16)
            ct = [singles.tile([128, QW], BF16, name=f"ct{t}")
                  for t in range(2 * NQC)]
            rcp = [singles.tile([128, NQC], F32, name=f"rcp{t}")
                   for t in range(2 * NQC)]

            def emit_proj(o_sb, w_sb, h, qc, bcol, pool, tag):
                win = slice(QW * qc, QW * qc + QW)
                ps = pool.tile([128, QW], F32, name=f"pp{tag}{h}{qc}", tag=tag)
                for dc in range(NDC):
                    nc.tensor.matmul(ps[:], w_sb[:, dc, 128 * h:128 * h + 128],
                                     xt_sb[:, dc, qc, :],
                                     start=(dc == 0), stop=(dc == NDC - 1))
                nc.vector.tensor_scalar_add(
                    o_sb[:, h, win], ps[:], bqk_sb[:, bcol:bcol + 1])

            def emit_vproj(lb, pool, tag):
                ps = pool.tile([128, QW], F32, name=f"pv{lb}", tag=tag)
                for dc in range(NDC):
                    nc.tensor.matmul(
                        ps[:, 0:256],
                        xt_sb[:, dc, lb // 4, 128 * (lb % 4):128 * (lb % 4) + 128],
                        wv_sb[:, dc, :],
                        start=(dc == 0), stop=(dc == NDC - 1))
                nc.vector.tensor_add(v_sb[:, lb, :], ps[:, 0:256], bv_sb[:])

            # ---- per-unit state ----
            # unit t = (qc, h) with t = 2*qc + h; scores/exp of unit t overlap
            # ctx of unit t-1 (cross-iteration software pipeline).
            smf = {}   # (t, kk) -> [128,512] pair-fold
            sm2 = {}
            sm3 = {}
            sm4 = {}

            def emit_scores_pair(t, qc, h, kk):
                win = slice(QW * qc, QW * qc + QW)
                ps_s = pss_pool.tile([128, 1024], F32,
                                     name=f"ps_s{t}_{kk}", tag="pss")
                k0 = 256 * kk
                nc.tensor.matmul(ps_s[:, 0:512], kt_sb[:, h, k0:k0 + 128],
                                 qt_sb[:, h, win], start=True, stop=True)
                nc.tensor.matmul(ps_s[:, 512:1024],
                                 kt_sb[:, h, k0 + 128:k0 + 256],
                                 qt_sb[:, h, win], start=True, stop=True)
                at = attnp.tile([128, 1024], BF16, name=f"at{t}_{kk}",
                                tag="attn")
                nc.scalar.activation(at[:], ps_s[:], AF.Exp, scale=SCALE)
                sm = smfp.tile([128, 512], BF16, name=f"sm{t}_{kk}", tag="sm")
                # gpsimd takes ONE self-contained subtree per unit (kk 0-1
                # lvl1 + their lvl2).  Larger gpsimd shares regressed hard:
                # its TT is ~2x slower, and the rowsum chain ends in a PE
                # rank-1 matmul sitting in the strict PE FIFO — any gpsimd
                # lag there stalls the ENTIRE matmul stream.
                fe = nc.gpsimd if kk in (0, 1) else nc.vector
                fe.tensor_add(sm[:], at[:, 0:512], at[:, 512:1024])
                smf[(t, kk)] = sm
                # second fold level: 4 rank-1 inputs per unit
                if kk % 2 == 1:
                    s2 = sm2p.tile([128, 512], BF16, name=f"s2{t}_{kk}",
                                   tag="sm2")
                    fe.tensor_add(s2[:], smf[(t, kk - 1)][:], sm[:])
                    sm2[(t, kk // 2)] = s2
                return at

            def emit_fold3(t, jj):
                # third fold level (DVE): sm3 = sm2[2jj] + sm2[2jj+1]
                s3 = sm3p.tile([128, 512], BF16, name=f"s3{t}_{jj}",
                               tag="sm3")
                nc.vector.tensor_add(s3[:], sm2[(t, 2 * jj)][:],
                                     sm2[(t, 2 * jj + 1)][:])
                sm3[(t, jj)] = s3

            def emit_fold4(t):
                # fourth fold level: one [128,512] tile holds the full
                # 16-block key fold -> a SINGLE rank-1 matmul per unit.
                s4 = sm3p.tile([128, 512], BF16, name=f"s4{t}", tag="sm3")
                nc.vector.tensor_add(s4[:], sm3[(t, 0)][:], sm3[(t, 1)][:])
                sm4[t] = s4

            def emit_rank1(t, ps_r):
                nc.tensor.matmul(ps_r[:], ones_sb[:], sm4[t][:],
                                 start=True, stop=True)

            def emit_rank1T(t, qq, ps_rc):
                # rowsum directly in column layout [128,4]: tiny
                # stationary-sm4 matmul per query window. Used for the last
                # two units only, where the DRAM-bounce transpose latency
                # would sit on the critical tail.
                nc.tensor.matmul(ps_rc[:, qq:qq + 1],
                                 sm4[t][:, 128 * qq:128 * qq + 128],
                                 ones_sb[:], start=True, stop=True)

            def emit_ctx_pair(st, kk):
                t, h, at_tiles, ps_c, _ = st
                hs = slice(128 * h, 128 * h + 128)
                at = at_tiles[kk]
                nc.tensor.matmul(ps_c[:], v_sb[:, 2 * kk, hs], at[:, 0:512],
                                 start=(kk == 0), stop=False)
                nc.tensor.matmul(ps_c[:], v_sb[:, 2 * kk + 1, hs],
                                 at[:, 512:1024], start=False,
                                 stop=(kk == NKB // 2 - 1))

            def emit_recip(t, ps_rc):
                nc.vector.reciprocal(rcp[t][:], ps_rc[:])

            rcol_t = {}

            def emit_rowsum_bounce(t, ps_r):
                # ps_r [1,512] row -> DRAM -> transposed read [128,4]; the
                # reciprocal is emitted separately TWO slots later so the
                # DVE FIFO never parks on the bounce's ~2-3us latency.
                # row copy on DVE (short PE dep; keeps the saturated ACT
                # queue out of the rowsum chain); bounce DMAs on gp, which
                # carries no bulk transfers mid-stream
                rrow = rrp.tile([1, QW], F32, name=f"rr{t}", tag="rr")
                nc.vector.tensor_copy(rrow[:], ps_r[:])
                rd = drp.tile([1, QW], F32, name=f"rd{t}", tag="rd")
                gp.dma_start(rd[:], rrow[:])
                rda = rd[:]
                rcol = rclp.tile([128, NQC], F32, name=f"rc{t}", tag="rc")
                gp.dma_start(rcol[:],
                             bass.AP(tensor=rda.tensor, offset=rda.offset,
                                     ap=[[1, 128], [128, NQC]]))
                rcol_t[t] = rcol

            def emit_rowsum_recip(t):
                nc.vector.reciprocal(rcp[t][:], rcol_t[t][:])

            def emit_ct_copy(t, ps_c, on_act=False):
                if on_act:
                    nc.scalar.activation(ct[t][:], ps_c[:], AF.Copy)
                else:
                    nc.vector.tensor_copy(ct[t][:], ps_c[:])

            def emit_outproj_window(qc, qq, pool, tag, tmp_act=False):
                qsl = slice(128 * qq, 128 * qq + 128)
                ps_y = pool.tile([128, 2, 512], F32, name=f"py{qc}_{qq}",
                                 tag=tag)
                nc.tensor.matmul(ps_y[:, 0, :], ct[2 * qc][:, qsl],
                                 wo_sb[:, 0, :], start=True, stop=True)
                nc.tensor.matmul(ps_y[:, 1, :], ct[2 * qc + 1][:, qsl],
                                 wo_sb[:, 1, :], start=True, stop=True)
                tmp = tmpp.tile([128, 512], F32, name=f"tm{qc}_{qq}",
                                tag="tmp")
                if tmp_act:
                    nc.scalar.activation(tmp[:], ps_y[:, 1, :], AF.Copy,
                                         scale=rcp[2 * qc + 1][:, qq:qq + 1])
                else:
                    nc.vector.tensor_scalar_mul(
                        tmp[:], ps_y[:, 1, :], rcp[2 * qc + 1][:, qq:qq + 1])
                ysb = yp.tile([128, 512], F32, name=f"ysb{qc}_{qq}",
                              tag="ysb")
                nc.vector.scalar_tensor_tensor(
                    ysb[:], ps_y[:, 0, :], rcp[2 * qc][:, qq:qq + 1], tmp[:],
                    ALU.mult, ALU.add)
                # y windows go on sync (idle mid-stream; sc would delay the
                # ACT exp queue); the final group splits each window into
                # half-partition transfers on two queues so the tail
                # transfers run in parallel instead of 1.9us serial each
                ywin = y_d[QW * qc + 128 * qq:QW * qc + 128 * qq + 128, :]
                if tmp_act:
                    q0, q1 = (sy, gp) if qq % 2 == 0 else (gp, sy)
                    q0.dma_start(ywin[0:64, :], ysb[0:64, :])
                    q1.dma_start(ywin[64:128, :], ysb[64:128, :])
                else:
                    sy.dma_start(ywin, ysb[:])

            # ---- prologue: just K/Q head 0 window 0; the rest of the
            # projections chase the x DMA stream from inside units 0-1 ----
            emit_proj(kt_sb, wk_sb, 0, 0, 2, pss_pool, "pss")
            emit_proj(qt_sb, wq_sb, 0, 0, 0, pss_pool, "pss")

            # deferred PE work fed into the units' kk slots
            # t=0 fillers use the (still idle) psc bank; t=1 fillers use pss.
            def P(o, w, h, qc, bc, pool, tag):
                return lambda: emit_proj(o, w, h, qc, bc, pool, tag)

            fillers_t0 = [
                P(kt_sb, wk_sb, 0, 1, 2, psc_pool, "psc"),
                P(kt_sb, wk_sb, 0, 2, 2, psc_pool, "psc"),
                P(qt_sb, wq_sb, 0, 1, 0, psc_pool, "psc"),
                P(kt_sb, wk_sb, 0, 3, 2, psc_pool, "psc"),
                P(qt_sb, wq_sb, 0, 2, 0, psc_pool, "psc"),
                P(kt_sb, wk_sb, 1, 0, 3, psc_pool, "psc"),
                P(qt_sb, wq_sb, 1, 0, 1, psc_pool, "psc"),
                P(kt_sb, wk_sb, 1, 1, 3, psc_pool, "psc"),
            ]
            fillers_t1 = [
                P(qt_sb, wq_sb, 0, 3, 0, pss_pool, "pss"),
                P(kt_sb, wk_sb, 1, 2, 3, pss_pool, "pss"),
                P(kt_sb, wk_sb, 1, 3, 3, pss_pool, "pss"),
                P(qt_sb, wq_sb, 1, 1, 1, pss_pool, "pss"),
                P(qt_sb, wq_sb, 1, 2, 1, pss_pool, "pss"),
                P(qt_sb, wq_sb, 1, 3, 1, pss_pool, "pss"),
            ]

            pending_outproj = []   # list of (qc, qq)
            to_pend = []           # windows whose rcp chain is still landing
            prev = None
            T_LAST = 2 * NQC - 1
            for t in range(2 * NQC):
                qc, h = t // 2, t % 2
                ps_c = psc_pool.tile([128, QW], F32, name=f"ps_c{t}",
                                     tag="psc")
                ps_r = (psr_pool.tile([1, QW], F32, name=f"ps_r{t}",
                                      tag="psr") if t < 6 else None)
                at_tiles = []
                sfirst = (t >= 6)   # feed ACT early near the end
                for kk in range(NKB // 2):
                    if sfirst:
                        at_tiles.append(emit_scores_pair(t, qc, h, kk))
                    if prev is not None:
                        emit_ctx_pair(prev, kk)
                        # previous unit's rowsum: rank-1 + drain, placed
                        # late enough that the (DVE) fold chain has finished
                        if prev[0] < 6:
                            if kk == 2:
                                emit_rank1(prev[0], prev[4])
                            elif kk == 3:
                                emit_rowsum_bounce(prev[0], prev[4])
                            elif kk == 5:
                                emit_rowsum_recip(prev[0])
                        else:
                            if kk == 2:
                                prc6 = psr_pool.tile(
                                    [128, NQC], F32, name="prc6", tag="psr")
                                emit_rank1T(6, 0, prc6)
                                emit_rank1T(6, 1, prc6)
                            elif kk == 3:
                                emit_rank1T(6, 2, prc6)
                                emit_rank1T(6, 3, prc6)
                                emit_recip(6, prc6)
                        if kk == 6 and to_pend:
                            pending_outproj.extend(to_pend)
                            to_pend = []
                    if t == 0:
                        emit_vproj(2 * kk, pss_pool, "pss")
                        emit_vproj(2 * kk + 1, pss_pool, "pss")
                        fillers_t0[kk]()
                    elif t == 1 and kk < len(fillers_t1):
                        fillers_t1[kk]()
                    if kk % 2 == 1 and pending_outproj:
                        emit_outproj_window(*pending_outproj.pop(0),
                                            psy_pool, "psy")
                    if not sfirst:
                        at_tiles.append(emit_scores_pair(t, qc, h, kk))
                    if kk == 4:
                        emit_fold3(t, 0)
                emit_fold3(t, 1)
                emit_fold4(t)
                if prev is not None:
                    # finish unit t-1: drain its ctx PSUM to bf16 SBUF
                    emit_ct_copy(prev[0], prev[3])
                    if prev[0] % 2 == 1:
                        # defer the windows to the NEXT unit's kk==6 slot so
                        # their rcp consumers never enter the DVE FIFO
                        # before the rowsum bounce+recip have resolved
                        to_pend = [((prev[0] - 1) // 2, qq)
                                   for qq in range(NQC)]
                prev = (t, h, at_tiles, ps_c, ps_r)

            # ---- drain: ctx of the last unit; its rowsum goes straight to
            # column layout on the PE (no DRAM bounce in the tail) ----
            ps_rc = psr_pool.tile([128, NQC], F32, name="ps_rc", tag="psr")
            for kk in range(NKB // 2):
                emit_ctx_pair(prev, kk)
                # windows deferred past the end of the main loop drain here
                if kk % 2 == 1 and pending_outproj:
                    emit_outproj_window(*pending_outproj.pop(0),
                                        psy_pool, "psy")
                if 2 <= kk <= 5:
                    emit_rank1T(T_LAST, kk - 2, ps_rc)
                    if kk == 5:
                        emit_recip(T_LAST, ps_rc)
            emit_ct_copy(prev[0], prev[3], on_act=True)
            # last window group (qc=3): rotate through pss (now idle) + psy
            for qq in range(NQC):
                # qq3 goes through pss (bufs=2): it then waits on window 0's
                # combine instead of window 1's, starting ~0.7us earlier
                if qq == 1:
                    emit_outproj_window(3, qq, psy_pool, "psy", tmp_act=True)
                else:
                    emit_outproj_window(3, qq, pss_pool, "pss", tmp_act=True)

    nc.compile()
    return nc


def _get_compiled():
    global _COMPILED
    if _COMPILED is None:
        _COMPILED = _build()
    return _COMPILED


def make_in_maps(x, Wq, bq, Wk, bk, Wv, bv, Wo):
    bf16 = ml_dtypes.bfloat16
    xT = {b: np.ascontiguousarray(x[b].T).astype(bf16) for b in range(B)}
    WqT, WkT, WvT, WoT = (np.ascontiguousarray(W.T) for W in (Wq, Wk, Wv, Wo))
    in_maps = []
    for c in range(NCORES):
        b = c // 2
        p = c % 2
        hs = slice(256 * p, 256 * p + 256)
        bqk = np.concatenate(
            [bq[hs].reshape(2, 128).T, bk[hs].reshape(2, 128).T],
            axis=1)
        in_maps.append({
            "xT": xT[b],
            "wqT": WqT[:, hs].astype(bf16),
            "wkT": WkT[:, hs].astype(bf16),
            "wvT": WvT[:, hs].astype(bf16),
            "woT": np.ascontiguousarray(WoT[hs, :]).astype(bf16),
            "bqk": np.ascontiguousarray(bqk, dtype=np.float32),
            "bv": bv[hs].reshape(1, 256).astype(np.float32).copy(),
        })
    return in_maps


def kernel(x, Wq, bq, Wk, bk, Wv, bv, Wo, bo):
    from concourse.bass_utils import run_bass_kernel_spmd

    x = np.asarray(x, np.float32)
    Wq, Wk, Wv, Wo = (np.asarray(w, np.float32) for w in (Wq, Wk, Wv, Wo))
    bq, bk, bv, bo = (np.asarray(b, np.float32) for b in (bq, bk, bv, bo))

    in_maps = make_in_maps(x, Wq, bq, Wk, bk, Wv, bv, Wo)
    nc = _get_compiled()
    try:
        # first execution is a discarded warmup (cold-start timing shakeout);
        # the returned result comes from the second execution
        run_bass_kernel_spmd(nc, in_maps, list(range(NCORES)))
        res = run_bass_kernel_spmd(nc, in_maps, list(range(NCORES)))
    except Exception:
        # one retry: transient device wedges usually clear on re-execution
        res = run_bass_kernel_spmd(nc, in_maps, list(range(NCORES)))
    y = np.empty((B, L, D), np.float32)
    for b in range(B):
        y[b] = res.results[2 * b]["y"] + res.results[2 * b + 1]["y"] + bo
    return y

